# revision 1
# baseline (speedup 1.0000x reference)
"""Trainium2 Bass kernel: MultiHeadSelfAttention (B=1, S=4096, D=512, H=8, DK=DV=64)
with fc_out applied twice.

Sharding: sequence-sharded across 8 cores (512 queries per core). Every core
receives the FULL keys/values (pre-transposed, bf16) and redundantly computes
the full K^T / V projections on-device (cheaper than an AllGather, whose entry
barrier + transfer measured ~100us); attention + the two output projections run
on the core's own 512-query chunk. Host concatenates the 8 output chunks.

Layout notes:
  - scores^T tiles [seq_k(128) x seq_q(512)] come out of PE via lhsT=K^T block,
    rhs=q^T. Both are zero-padded from d=64 to K=128 partitions: K=64 matmuls
    never trip the PE HAM activity monitor, pinning the clock to 1.2 GHz
    (measured); K=128 with zero rows sustains 2.4 GHz.
  - softmax denominator via a ones-column appended to each head's V (stride
    65): attn@V gives [65, 512] per head = output^T rows + exp-sum row.
  - the two fc_out applications are folded on the host (W2 = Wo@Wo,
    b2 = bo@Wo + bo): one device fc pass, half the matmuls, no serial chain.
  - output returned TRANSPOSED ([D, CH]); host un-transposes. No PE
    transposes or tail copies; fc partials (k=0..2) overlap head 7's
    normalization chain, then per-128-dim-chunk bias+DMA on disjoint tiles
    split across ACT/DVE + sync/gpsimd so nothing serializes cross-engine.
  - 20 throwaway matmuls lead the PE stream: the PE runs at ~1.2 GHz until
    ~3us of continuous execution, and the first real matmul can't start
    before its DMA lands (~12us); the warmup ramps the clock meanwhile.
  - heads are ACT-paced (exp chain ~16.1us/head vs ~13.65us of PE work), so
    K-pair projections drip into the PE slack inside heads 1-6.
"""
import sys, functools
sys.path.insert(0, "/opt/trn_rl_repo")
if "/root/.axon_site" not in sys.path:
    sys.path.insert(0, "/root/.axon_site")
import numpy as np
import ml_dtypes

import concourse.bass as bass
import concourse.tile as tile
from concourse import bacc, mybir, masks
from concourse.bass_utils import run_bass_kernel_spmd

NCORES = 8
S, D, H, DK = 4096, 512, 8, 64
CH = S // NCORES            # 512 sequence rows per core
VW = H * (DK + 1)           # 520: v row width incl. ones columns
JT = S // 128               # 32 seq_k tiles
CHUNK = 3                   # j-tiles per exp batch ([128,1536] psum, 3 banks x 2)

F32 = mybir.dt.float32
BF16 = mybir.dt.bfloat16
EXP = mybir.ActivationFunctionType.Exp


def _build_program():
    nc = bacc.Bacc("TRN2", target_bir_lowering=False, debug=False,
                   num_devices=NCORES)

    xqT = nc.dram_tensor("xqT", [D, CH], BF16, kind="ExternalInput")
    keysT = nc.dram_tensor("keysT", [D, S], BF16, kind="ExternalInput")
    valsT = nc.dram_tensor("valsT", [D, S], BF16, kind="ExternalInput")
    Wq = nc.dram_tensor("Wq", [D, D], BF16, kind="ExternalInput")
    Wk = nc.dram_tensor("Wk", [D, D], BF16, kind="ExternalInput")
    Wv = nc.dram_tensor("Wv", [D, D], BF16, kind="ExternalInput")
    # host folds the two fc_out applications: W2 = Wo@Wo, b2 = bo@Wo + bo
    Wo = nc.dram_tensor("Wo", [D, D], BF16, kind="ExternalInput")
    bo = nc.dram_tensor("bo", [D], F32, kind="ExternalInput")
    # output^T: [D, CH] bf16 (host transposes + upcasts; the 0.4% bf16
    # rounding is well inside the error budget and halves the final DMA)
    yT = nc.dram_tensor("yT", [D, CH], BF16, kind="ExternalOutput")

    with tile.TileContext(nc) as tc:
        with tc.tile_pool(name="persist", bufs=1) as pp, \
             tc.tile_pool(name="kv", bufs=1) as kvp:

            Wo_sb = pp.tile([128, 2048], BF16, tag="wo")
            Wk_sb = pp.tile([128, 2048], BF16, tag="wk")
            Wv_sb = pp.tile([128, 2048], BF16, tag="wv")
            # two bias tiles, one per bias engine (ACT / DVE), so the tail
            # bias-adds share no tiles across engines
            boA = pp.tile([128, 2], F32, tag="boA")
            boB = pp.tile([128, 2], F32, tag="boB")
            ones64b = pp.tile([1, 64], BF16, tag="on")
            warm_sb = pp.tile([128, 256], BF16, tag="warm")
            o2p = [pp.tile([128, 512], BF16, tag=f"o2{m}", name=f"o2_{m}")
                   for m in range(4)]
            # q^T per head: even heads rows 0-63 (zeros below), odd heads rows
            # 64-127 (zeros above) - matches the packed K^T pair layout
            qTz_sb = pp.tile([128, H * 512], BF16, tag="qt")
            # attention output^T, one tile per head pair so fc partials can
            # start as soon as a pair completes
            attTp = [pp.tile([128, 512], BF16, tag=f"att{p}", name=f"attT{p}")
                     for p in range(4)]
            # K^T packed head pairs: head 2p on rows 0-63, head 2p+1 on 64-127;
            # the zero padding that keeps scores at K=128 lives in qTz instead
            KTp = [kvp.tile([128, S], BF16, tag=f"kt{p}", name=f"KT{p}")
                   for p in range(H // 2)]
            # V natural [seq, head-stripes of 65 (64 + ones col)]
            V_sb = kvp.tile([128, JT * VW], BF16, tag="v")

            # zero pads + ones columns on gpsimd (keeps DVE free)
            nc.vector.memset(warm_sb[:], 0.0)
            nc.vector.memset(qTz_sb[:], 0.0)
            nc.vector.memset(ones64b[:], 1.0)
            nc.gpsimd.memset(
                V_sb[:].rearrange("p (j h x) -> p j h x", j=JT, h=H, x=DK + 1)
                [:, :, :, DK:DK + 1], 1.0)

            with tc.tile_pool(name="kstage", bufs=1) as ksp, \
                 tc.tile_pool(name="pt", bufs=6) as ptp, \
                 tc.tile_pool(name="rc", bufs=2) as rcp, \
                 tc.tile_pool(name="ps_av", bufs=1, space="PSUM") as psav:

                def q_proj(pool):
                    for m in range(4):
                        ps = pool.tile([128, 512], F32, tag="bg", name=f"qp{m}")
                        for k in range(4):
                            nc.tensor.matmul(
                                ps[:], lhsT=Wq_sb[:, 512 * k + 128 * m:512 * k + 128 * m + 128],
                                rhs=xqT_sb[:, 512 * k:512 * k + 512],
                                start=(k == 0), stop=(k == 3))
                        nc.vector.tensor_copy(
                            qTz_sb[0:64, 512 * (2 * m):512 * (2 * m) + 512], ps[0:64, :])
                        nc.vector.tensor_copy(
                            qTz_sb[64:128, 512 * (2 * m + 1):512 * (2 * m + 1) + 512],
                            ps[64:128, :])

                def v_proj_group(j, pool):
                    ps = pool.tile([128, 512], F32, tag="bg", name=f"vp{j}")
                    vt, jj = vst[j // 4], j % 4
                    for k in range(4):
                        nc.tensor.matmul(
                            ps[:], lhsT=vt[:, 512 * k + 128 * jj:512 * k + 128 * jj + 128],
                            rhs=Wv_sb[:, 512 * k:512 * k + 512],
                            start=(k == 0), stop=(k == 3))
                    dst = V_sb[:, VW * j:VW * j + VW].rearrange(
                        "p (h x) -> p h x", h=H, x=DK + 1)[:, :, 0:DK]
                    nc.vector.tensor_copy(
                        dst, ps[:].rearrange("p (h x) -> p h x", h=H, x=DK))

                def k_proj_group(m, sc, pool, tag, copy_eng=None):
                    ps = pool.tile([128, 512], F32, tag=tag, name=f"kp{m}_{sc}")
                    for k in range(4):
                        nc.tensor.matmul(
                            ps[:], lhsT=Wk_sb[:, 512 * k + 128 * m:512 * k + 128 * m + 128],
                            rhs=kst0[sc][:, 512 * k:512 * k + 512],
                            start=(k == 0), stop=(k == 3))
                    dst = KTp[m][:, 512 * sc:512 * sc + 512]
                    if copy_eng == "act":
                        # ACT is idle between heads; copying there keeps the
                        # single kproj psum bank draining without queueing
                        # behind the DVE normalization chain
                        nc.scalar.copy(dst, ps[:])
                    else:
                        nc.vector.tensor_copy(dst, ps[:])

                def attention_head(h, pool_sc, chunk, drip=None):
                    q_ap = qTz_sb[:, 512 * h:512 * h + 512]
                    av = psav.tile([65, 512], F32, tag="av", name=f"av{h}")

                    def attn_v(js, pt):
                        for i, j in enumerate(js):
                            nc.tensor.matmul(
                                av[:],
                                lhsT=V_sb[:, VW * j + 65 * h:VW * j + 65 * h + 65],
                                rhs=pt[:, 512 * i:512 * i + 512],
                                start=(j == 0), stop=(j == JT - 1))

                    pend = None  # attn@V lags one chunk so scores stay ahead of ACT
                    for c in range((JT + chunk - 1) // chunk):
                        js = list(range(chunk * c, min(chunk * c + chunk, JT)))
                        ps = pool_sc.tile([128, 512 * chunk], F32, tag="sc",
                                          name=f"sc{h}_{c}")
                        pt = ptp.tile([128, 512 * chunk], BF16, tag="pt",
                                      name=f"pt{h}_{c}")
                        for i, j in enumerate(js):
                            nc.tensor.matmul(
                                ps[:, 512 * i:512 * i + 512],
                                lhsT=KTp[h // 2][:, 128 * j:128 * j + 128],
                                rhs=q_ap, start=True, stop=True)
                        w = 512 * len(js)
                        nc.scalar.activation(pt[:, 0:w], ps[:, 0:w], EXP, scale=0.125)
                        if drip is not None:
                            drip(c)
                        if pend is not None:
                            attn_v(*pend)
                        pend = (js, pt)
                    last_pt = pend[1]
                    attn_v(*pend)
                    hp, hl = h // 2, h % 2
                    att_dst = attTp[hp][64 * hl:64 * hl + 64, :]
                    if h == 7:
                        # critical-path tail: read av psum directly, broadcast
                        # the denominator via a K=1 matmul on the (idle) PE
                        rtmpb = rcp.tile([1, 512], BF16, tag="rt", name="rt7")
                        nc.vector.tensor_copy(rtmpb[:], av[64:65, :])
                        rbb = pskp.tile([64, 512], F32, tag="kp", name="rbb7")
                        nc.tensor.matmul(rbb[:], lhsT=ones64b[:], rhs=rtmpb[:],
                                         start=True, stop=True)
                        rb2 = rcp.tile([64, 512], F32, tag="rb2", name="rb27")
                        nc.vector.reciprocal_approx_fast(out=rb2[:], in_=rbb[:])
                        nc.vector.tensor_mul(att_dst, av[0:64, :], rb2[:])
                        return last_pt
                    # copy psum accumulator out immediately so the bank frees
                    avc = rcp.tile([65, 512], F32, tag="avc", name=f"avc{h}")
                    rbc = rcp.tile([64, 512], F32, tag="rb", name=f"rb{h}")
                    rtmp = rcp.tile([1, 512], F32, tag="rt", name=f"rt{h}")
                    nc.vector.tensor_copy(avc[:], av[:])
                    nc.vector.tensor_copy(rtmp[:], av[64:65, :])
                    rb2 = rcp.tile([64, 512], F32, tag="rb2", name=f"rb2{h}")
                    nc.gpsimd.partition_broadcast(rbc[:], rtmp[:])
                    nc.vector.reciprocal_approx_fast(out=rb2[:], in_=rbc[:])
                    nc.vector.tensor_mul(att_dst, avc[0:64, :], rb2[:])

                # ---- scope A: q proj, K0, head 0 with V-proj dripped in ----
                pfx = tc.tile_pool(name="xin", bufs=1)
                xp = pfx.__enter__()
                scA = tc.tile_pool(name="ps_scA", bufs=2, space="PSUM")
                pssc2 = scA.__enter__()
                bgp_cm = tc.tile_pool(name="ps_bg", bufs=2, space="PSUM")
                bgp = bgp_cm.__enter__()

                Wq_sb = xp.tile([128, 2048], BF16, tag="wq")
                xqT_sb = xp.tile([128, 2048], BF16, tag="xq")
                # one tile per 512-seq chunk (layout [p, (k=4, 512)]) so each
                # projection group depends on exactly its own chunk's DMA
                # instead of the whole-staging-tile writer set
                vst = [xp.tile([128, 2048], BF16, tag=f"vs{c}", name=f"vst{c}")
                       for c in range(8)]
                kst0 = [ksp.tile([128, 2048], BF16, tag=f"ks{c}", name=f"kst{c}")
                        for c in range(8)]
                # split first loads so q_proj can begin on the first halves;
                # K/V staging issues from otherwise-idle engine queues so the
                # sync engine's serial DMA-issue cost doesn't pace arrivals
                xq_d = xqT_sb[:].rearrange("p (k s) -> p k s", k=4)
                xq_s = xqT.ap().rearrange("(k p) s -> p k s", p=128)
                wq_d = Wq_sb[:].rearrange("p (k n) -> p k n", k=4)
                wq_s = Wq.ap().rearrange("(k p) n -> p k n", p=128)
                nc.sync.dma_start(xq_d[:, 0:2, :], xq_s[:, 0:2, :])
                nc.sync.dma_start(wq_d[:, 0:2, :], wq_s[:, 0:2, :])
                nc.sync.dma_start(xq_d[:, 2:4, :], xq_s[:, 2:4, :])
                nc.sync.dma_start(wq_d[:, 2:4, :], wq_s[:, 2:4, :])
                nc.sync.dma_start(
                    Wk_sb[:].rearrange("p (k n) -> p k n", k=4),
                    Wk.ap().rearrange("(k p) n -> p k n", p=128))
                nc.sync.dma_start(
                    Wv_sb[:].rearrange("p (k n) -> p k n", k=4),
                    Wv.ap().rearrange("(k p) n -> p k n", p=128))
                # K/V chunks interleaved to match head-0's consumption order:
                # chunk c of head 0 consumes kst c+1 and vst c//2
                ks_ = keysT.ap().rearrange("(k p) s -> p k s", p=128)
                vs_ = valsT.ap().rearrange("(k p) s -> p k s", p=128)
                order = [(0, 0), (0, 1), (1, 0), (0, 2), (0, 3), (1, 1),
                         (0, 4), (0, 5), (1, 2), (0, 6), (0, 7), (1, 3),
                         (1, 4), (1, 5), (1, 6), (1, 7)]
                for which, ci in order:
                    tl, src = ((kst0, ks_) if which == 0 else (vst, vs_))
                    nc.sync.dma_start(
                        tl[ci][:].rearrange("p (k s) -> p k s", k=4),
                        src[:, :, 512 * ci:512 * ci + 512])
                nc.sync.dma_start(
                    Wo_sb[:].rearrange("p (k n) -> p k n", k=4),
                    Wo.ap().rearrange("(k p) n -> p k n", p=128))
                bo_src = bo.ap().rearrange("(m p) -> p m", p=128)
                nc.sync.dma_start(boA[:], bo_src[:, 0:2])
                nc.sync.dma_start(boB[:], bo_src[:, 2:4])

                # PE p-state warmup: matmuls run at ~1.2 GHz until the PE has
                # been continuously busy ~3us, and the first real matmul can't
                # start until its DMA lands (~12us). These throwaway 256-col
                # matmuls are emitted BEFORE q_proj so they unconditionally
                # lead the PE stream, ramping the clock while DMA streams in.
                wrm_cm = tc.tile_pool(name="ps_warm", bufs=1, space="PSUM")
                wrm = wrm_cm.__enter__()
                warm_ps = wrm.tile([64, 256], F32, tag="w", name="warm_ps")
                for _ in range(20):
                    nc.tensor.matmul(
                        warm_ps[:], lhsT=warm_sb[:, 0:64],
                        rhs=warm_sb[:, 0:256], start=True, stop=True)
                wrm_cm.__exit__(None, None, None)

                q_proj(bgp)
                k_proj_group(0, 0, bgp, "bg")

                def drip_kv(c):
                    # K-pair-0 groups dripped just ahead of the chunks that
                    # need them (chunk c+1 needs sc <= (2c+3)//4 <= c+1), so
                    # the PE isn't parked on not-yet-loaded kst chunks
                    if c + 1 < 8:
                        k_proj_group(0, c + 1, bgp, "bg")
                    for j in (2 * c, 2 * c + 1):
                        if j < JT:
                            v_proj_group(j, bgp)

                attention_head(0, pssc2, 2, drip_kv)

                bgp_cm.__exit__(None, None, None)
                scA.__exit__(None, None, None)
                pfx.__exit__(None, None, None)

                # ---- scope B: heads 1-7; K pair m batched before head 2m;
                # fused fc (W2 = Wo@Wo folded on host) emitted in-scope so its
                # psum accumulators carve the scores-pool banks with no
                # scope-exit barrier: k=0..2 partials fill the PE gap while
                # head 7's normalization chain finishes attTp[3] ----
                with tc.tile_pool(name="ps_sc", bufs=2, space="PSUM") as pssc, \
                     tc.tile_pool(name="ps_kp", bufs=1, space="PSUM") as pskp:
                    # heads are ACT-paced (exp chain ~16.1us vs PE ~13.65us),
                    # so K pair m's 8 proj groups drip into the PE slack of
                    # heads 2m-1 (late chunks) and 2m (alternating chunks)
                    # instead of running as pure-PE batches between heads
                    sched = {h: {} for h in range(1, 8)}
                    for m in (1, 2, 3):
                        for i in range(4):
                            sched[2 * m - 1][7 + i] = (m, i)
                            sched[2 * m][2 + 2 * i] = (m, 4 + i)

                    def make_drip_k(h):
                        hs = sched.get(h, {})
                        if not hs:
                            return None
                        def drip(c):
                            if c in hs:
                                m, sc = hs[c]
                                k_proj_group(m, sc, pskp, "kp")
                        return drip

                    for h in range(1, 7):
                        attention_head(h, pssc, CHUNK, make_drip_k(h))
                    gate_pt = attention_head(7, pssc, CHUNK)

                    fcA = pssc.tile([128, 1536], F32, tag="sc", name="fcA")
                    fcB = pssc.tile([128, 1536], F32, tag="sc", name="fcB")
                    # gate the fc partials on head 7's last exp chunk: without
                    # this they become ready early and stuff the PE hardware
                    # queue ahead of head 7's final attn@V matmuls, delaying
                    # the whole normalization chain
                    nc.vector.tensor_copy(fcA[0:1, 0:1], gate_pt[0:1, 0:1])
                    nc.vector.tensor_copy(fcB[0:1, 0:1], gate_pt[0:1, 0:1])
                    fct = [fcA[:, 0:512], fcA[:, 512:1024],
                           fcB[:, 0:512], fcB[:, 512:1024]]
                    yT_d = yT.ap().rearrange("(m p) f -> p m f", m=4, p=128)
                    for m in range(4):
                        for k in range(3):
                            nc.tensor.matmul(
                                fct[m], lhsT=Wo_sb[:, 512 * k + 128 * m:512 * k + 128 * m + 128],
                                rhs=attTp[k][:], start=(k == 0), stop=False,
                                skip_group_check=True)
                    for m in range(4):
                        nc.tensor.matmul(
                            fct[m], lhsT=Wo_sb[:, 512 * 3 + 128 * m:512 * 3 + 128 * m + 128],
                            rhs=attTp[3][:], start=False, stop=True,
                            skip_group_check=True)
                    # biases paired same-engine per psum tile (fcA -> ACT,
                    # fcB -> DVE) with per-engine bias tiles: the tail stages
                    # share no tiles across engines, so nothing serializes
                    nc.scalar.add(o2p[0][:], fct[0], boA[:, 0:1])
                    nc.scalar.add(o2p[1][:], fct[1], boA[:, 1:2])
                    nc.vector.tensor_scalar_add(o2p[2][:], fct[2], boB[:, 0:1])
                    nc.vector.tensor_scalar_add(o2p[3][:], fct[3], boB[:, 1:2])
                    for m, eng in ((0, nc.sync), (1, nc.sync),
                                   (2, nc.gpsimd), (3, nc.gpsimd)):
                        eng.dma_start(yT_d[:, m, :], o2p[m][:])

    nc.compile()
    return nc


@functools.lru_cache(maxsize=1)
def _get_program():
    return _build_program()


def _make_in_maps(queries, keys, values, Wq, Wk, Wv, Wo, bo):
    q = np.asarray(queries, np.float32).reshape(S, D)
    kT = np.ascontiguousarray(np.asarray(keys, np.float32).reshape(S, D).T
                              ).astype(ml_dtypes.bfloat16)
    vT = np.ascontiguousarray(np.asarray(values, np.float32).reshape(S, D).T
                              ).astype(ml_dtypes.bfloat16)
    Wq = np.ascontiguousarray(np.asarray(Wq, np.float32)).astype(ml_dtypes.bfloat16)
    Wk = np.ascontiguousarray(np.asarray(Wk, np.float32)).astype(ml_dtypes.bfloat16)
    Wv = np.ascontiguousarray(np.asarray(Wv, np.float32)).astype(ml_dtypes.bfloat16)
    # fold the two fc_out applications into one: y = a@(Wo@Wo) + (bo@Wo + bo)
    Wo64 = np.asarray(Wo, np.float64)
    bo64 = np.asarray(bo, np.float64)
    Wo = np.ascontiguousarray(Wo64 @ Wo64).astype(ml_dtypes.bfloat16)
    bo = np.ascontiguousarray(bo64 @ Wo64 + bo64).astype(np.float32)
    in_maps = []
    for c in range(NCORES):
        in_maps.append({
            "xqT": np.ascontiguousarray(q[c * CH:(c + 1) * CH].T).astype(ml_dtypes.bfloat16),
            "keysT": kT, "valsT": vT,
            "Wq": Wq, "Wk": Wk, "Wv": Wv, "Wo": Wo, "bo": bo,
        })
    return in_maps


def _run(in_maps, **kw):
    nc = _get_program()
    return run_bass_kernel_spmd(nc, in_maps, core_ids=list(range(NCORES)), **kw)


def kernel(queries, keys, values, Wq, Wk, Wv, Wo, bo):
    res = _run(_make_in_maps(queries, keys, values, Wq, Wk, Wv, Wo, bo))
    out = np.concatenate([res.results[c]["yT"].T.astype(np.float32) for c in range(NCORES)], axis=0)
    return out.reshape(1, S, D)


def run_traced(queries, keys, values, Wq, Wk, Wv, Wo, bo):
    """Like kernel() but with NTFF profiling; returns (output, BassKernelResults)."""
    import types
    import trn_agent_boot.trn_boot as _tb
    from concourse import bass_utils
    hook = _tb._ntff_profile_via_ctypes("/opt/axon/libaxon_pjrt.so")
    mod = types.ModuleType("antenv.axon_hooks")
    mod.get_axon_ntff_profile_hook = lambda: hook
    sys.modules["antenv.axon_hooks"] = mod
    bass_utils.upload_artifacts = lambda tmpdir: tmpdir
    res = _run(_make_in_maps(queries, keys, values, Wq, Wk, Wv, Wo, bo), trace=True)
    out = np.concatenate([res.results[c]["yT"].T.astype(np.float32) for c in range(NCORES)], axis=0)
    return out.reshape(1, S, D), res



# revision 3
# speedup vs baseline: 1.1051x; 1.1051x over previous
"""Trainium2 Bass kernel: MultiHeadSelfAttention (B=1, S=4096, D=512, H=8, DK=DV=64)
with fc_out applied twice.

Sharding: 2-way sequence x 4-way head-pair hybrid. Core c = (s, g) with
s = c//4, g = c%4 handles queries [2048s : 2048s+2048] for head pair g
(heads 2g, 2g+1):
  - Wq/Wk/Wv column-sharded by pair: each core projects only its pair's
    K^T/V over the full 4096 keys (6.8us + ~13us PE vs 27.3+27.3 when every
    core projected all 8 heads redundantly).
  - fc_out row-sharded: each core computes the partial y^T = W2[pair rows]^T
    @ att^T for its 2048 queries; the HOST sums the 4 pair-partials per
    sequence half and adds the (folded) bias. No collectives anywhere.
  - attention runs as 8 "virtual heads" (4 query chunks of 512 x head lo/hi),
    structurally identical to the old 8-real-head loop, so the ACT exp chain
    (~16us/vhead, the second wall) and the psum pipeline are unchanged.

Layout notes (inherited from the seq-sharded ancestor):
  - scores^T tiles [seq_k(128) x seq_q(512)] via lhsT=K^T-pair block,
    rhs=q^T slot. K^T packs head lo on rows 0-63, head hi on 64-127; q^T
    slots zero the complementary rows so K=128 matmuls never trip the PE
    HAM activity monitor (K=64 pins the clock to 1.2 GHz; K=128 sustains
    2.4 GHz).
  - softmax denominator via a ones-column appended to each head's V (stride
    65): attn@V gives [65, 512] per vhead = output^T rows + exp-sum row.
  - the two fc_out applications are folded on the host (W2 = Wo@Wo,
    b2 = bo@Wo + bo); bias is added on host after the partial sum, so the
    device does no bias work at all.
  - output returned TRANSPOSED ([D, 2048] bf16 partial); host sums and
    un-transposes. fc runs one dout-chunk at a time, dripped into the PE
    slack of the vhead after the one that completed the att chunk, so fc
    matmuls are always ready when the PE reaches them (wait-queue depth is
    4 and an unready head blocks it); only chunk 3's fc lands in the tail.
  - 20 throwaway matmuls lead the PE stream: the PE runs at ~1.2 GHz until
    ~3us of continuous execution, and the first real matmul can't start
    before its DMA lands (~12us); the warmup ramps the clock meanwhile.
  - vheads are ACT-paced (exp chain ~16.1us/vhead vs ~13.65us of PE work);
    K-pair groups 1-7 and all 32 V groups drip into vhead 0's chunks, fc
    into the later vheads' PE slack.
"""
import sys, functools
sys.path.insert(0, "/opt/trn_rl_repo")
if "/root/.axon_site" not in sys.path:
    sys.path.insert(0, "/root/.axon_site")
import numpy as np
import ml_dtypes

import concourse.bass as bass
import concourse.tile as tile
from concourse import bacc, mybir, masks
from concourse.bass_utils import run_bass_kernel_spmd

NCORES = 8
S, D, H, DK = 4096, 512, 8, 64
SEQW = 2          # sequence-way
HPW = 4           # head-pair-way
CH = S // SEQW    # 2048 queries per core
NV = CH // 512    # 4 query chunks -> 8 virtual heads (chunk x lo/hi)
VW = 2 * (DK + 1)           # 130: v row width for the pair incl. ones columns
JT = S // 128               # 32 seq_k tiles
CHUNK = 3                   # j-tiles per exp batch ([128,1536] psum, 3 banks x 2)

F32 = mybir.dt.float32
BF16 = mybir.dt.bfloat16
EXP = mybir.ActivationFunctionType.Exp


def _build_program():
    nc = bacc.Bacc("TRN2", target_bir_lowering=False, debug=False,
                   num_devices=NCORES)

    xqT = nc.dram_tensor("xqT", [D, CH], BF16, kind="ExternalInput")
    keysT = nc.dram_tensor("keysT", [D, S], BF16, kind="ExternalInput")
    valsT = nc.dram_tensor("valsT", [D, S], BF16, kind="ExternalInput")
    Wq = nc.dram_tensor("Wq", [D, 128], BF16, kind="ExternalInput")
    Wk = nc.dram_tensor("Wk", [D, 128], BF16, kind="ExternalInput")
    Wv = nc.dram_tensor("Wv", [D, 128], BF16, kind="ExternalInput")
    # host folds the two fc_out applications (W2 = Wo@Wo) and slices this
    # core's pair rows [128, D]; bias is host-side entirely
    W2 = nc.dram_tensor("W2", [128, D], BF16, kind="ExternalInput")
    # partial output^T: [D, CH] bf16; host sums the 4 pair-partials
    yT = nc.dram_tensor("yT", [D, CH], BF16, kind="ExternalOutput")
    yT_d = yT.ap().rearrange("(m p) f -> p m f", m=4, p=128)

    with tile.TileContext(nc) as tc:
        with tc.tile_pool(name="persist", bufs=1) as pp, \
             tc.tile_pool(name="kv", bufs=1) as kvp:

            W2_sb = pp.tile([128, 512], BF16, tag="w2")
            Wk_sb = pp.tile([128, 512], BF16, tag="wk")
            Wv_sb = pp.tile([128, 512], BF16, tag="wv")
            ones64b = pp.tile([1, 64], BF16, tag="on")
            warm_sb = pp.tile([128, 256], BF16, tag="warm")
            o2p = [pp.tile([128, 512], BF16, tag=f"o2{m}", name=f"o2_{m}")
                   for m in range(4)]
            # q^T per vhead: lo vheads (v%2==0) rows 0-63 (zeros below), hi
            # vheads rows 64-127 (zeros above) - matches the packed K^T pair
            qTz_sb = pp.tile([128, 2 * NV * 512], BF16, tag="qt")
            # attention output^T, one tile per query chunk so fc can start as
            # soon as a chunk's lo+hi vheads complete
            attTp = [pp.tile([128, 512], BF16, tag=f"att{p}", name=f"attT{p}")
                     for p in range(4)]
            # K^T pair-packed: head lo on rows 0-63, head hi on 64-127
            KTp = kvp.tile([128, S], BF16, tag="kt")
            # V natural [seq, pair-stripes of 65 (64 + ones col)]
            V_sb = kvp.tile([128, JT * VW], BF16, tag="v")

            # zero pads + ones columns on gpsimd (keeps DVE free)
            nc.vector.memset(warm_sb[:], 0.0)
            nc.vector.memset(qTz_sb[:], 0.0)
            nc.vector.memset(ones64b[:], 1.0)
            nc.gpsimd.memset(
                V_sb[:].rearrange("p (j h x) -> p j h x", j=JT, h=2, x=DK + 1)
                [:, :, :, DK:DK + 1], 1.0)

            with tc.tile_pool(name="kstage", bufs=1) as ksp, \
                 tc.tile_pool(name="pt", bufs=6) as ptp, \
                 tc.tile_pool(name="rc", bufs=2) as rcp, \
                 tc.tile_pool(name="ps_av", bufs=1, space="PSUM") as psav:

                def q_proj(qc, pool):
                    ps = pool.tile([128, 512], F32, tag="bg", name=f"qp{qc}")
                    for k in range(4):
                        nc.tensor.matmul(
                            ps[:], lhsT=Wq_sb[:, 128 * k:128 * k + 128],
                            rhs=xqT_sb[:, 2048 * k + 512 * qc:2048 * k + 512 * qc + 512],
                            start=(k == 0), stop=(k == 3))
                    nc.vector.tensor_copy(
                        qTz_sb[0:64, 512 * (2 * qc):512 * (2 * qc) + 512], ps[0:64, :])
                    nc.vector.tensor_copy(
                        qTz_sb[64:128, 512 * (2 * qc + 1):512 * (2 * qc + 1) + 512],
                        ps[64:128, :])

                def v_proj_group(j, pool, tag="bg"):
                    # out [seq-tile 128, pair-d 128]: lhsT = valsT chunk slice,
                    # rhs = Wv pair slice (N=128; LDW-bound but only 32 groups)
                    ps = pool.tile([128, 512], F32, tag=tag, name=f"vp{j}")
                    vt, jj = vst[j // 4], j % 4
                    for k in range(4):
                        nc.tensor.matmul(
                            ps[0:128, 0:128],
                            lhsT=vt[:, 512 * k + 128 * jj:512 * k + 128 * jj + 128],
                            rhs=Wv_sb[:, 128 * k:128 * k + 128],
                            start=(k == 0), stop=(k == 3))
                    dst = V_sb[:, VW * j:VW * j + VW].rearrange(
                        "p (h x) -> p h x", h=2, x=DK + 1)[:, :, 0:DK]
                    nc.vector.tensor_copy(
                        dst, ps[0:128, 0:128].rearrange("p (h x) -> p h x", h=2, x=DK))

                def k_proj_group(sc, pool, tag="bg"):
                    ps = pool.tile([128, 512], F32, tag=tag, name=f"kp{sc}")
                    for k in range(4):
                        nc.tensor.matmul(
                            ps[:], lhsT=Wk_sb[:, 128 * k:128 * k + 128],
                            rhs=kst0[sc][:, 512 * k:512 * k + 512],
                            start=(k == 0), stop=(k == 3))
                    nc.vector.tensor_copy(KTp[:, 512 * sc:512 * sc + 512], ps[:])

                def fc_one(qc, m, pool):
                    # one dout-chunk of the partial y^T for query chunk qc;
                    # single din step (this core's pair rows only)
                    ps = pool.tile([128, 512], F32, tag="aux", name=f"fc{qc}_{m}")
                    nc.tensor.matmul(
                        ps[:], lhsT=W2_sb[:, 128 * m:128 * m + 128],
                        rhs=attTp[qc][:], start=True, stop=True)
                    nc.vector.tensor_copy(o2p[m][:], ps[:])
                    eng = nc.sync if m % 2 == 0 else nc.gpsimd
                    eng.dma_start(yT_d[:, m, 512 * qc:512 * qc + 512], o2p[m][:])

                def attention_head(v, pool_sc, chunk, drip=None, aux_pool=None):
                    q_ap = qTz_sb[:, 512 * v:512 * v + 512]
                    av = psav.tile([65, 512], F32, tag="av", name=f"av{v}")
                    voff = 65 * (v % 2)

                    def attn_v(js, pt):
                        for i, j in enumerate(js):
                            nc.tensor.matmul(
                                av[:],
                                lhsT=V_sb[:, VW * j + voff:VW * j + voff + 65],
                                rhs=pt[:, 512 * i:512 * i + 512],
                                start=(j == 0), stop=(j == JT - 1))

                    pend = None  # attn@V lags one chunk so scores stay ahead of ACT
                    for c in range((JT + chunk - 1) // chunk):
                        js = list(range(chunk * c, min(chunk * c + chunk, JT)))
                        ps = pool_sc.tile([128, 512 * chunk], F32, tag="sc",
                                          name=f"sc{v}_{c}")
                        pt = ptp.tile([128, 512 * chunk], BF16, tag="pt",
                                      name=f"pt{v}_{c}")
                        for i, j in enumerate(js):
                            nc.tensor.matmul(
                                ps[:, 512 * i:512 * i + 512],
                                lhsT=KTp[:, 128 * j:128 * j + 128],
                                rhs=q_ap, start=True, stop=True)
                        w = 512 * len(js)
                        nc.scalar.activation(pt[:, 0:w], ps[:, 0:w], EXP, scale=0.125)
                        if drip is not None:
                            drip(c)
                        if pend is not None:
                            attn_v(*pend)
                        pend = (js, pt)
                    last_pt = pend[1]
                    attn_v(*pend)
                    qc, hl = v // 2, v % 2
                    att_dst = attTp[qc][64 * hl:64 * hl + 64, :]
                    if v == 2 * NV - 1:
                        # critical-path tail: read av psum directly, broadcast
                        # the denominator via a K=1 matmul on the (idle) PE
                        rtmpb = rcp.tile([1, 512], BF16, tag="rt", name="rt7")
                        nc.vector.tensor_copy(rtmpb[:], av[64:65, :])
                        rbb = aux_pool.tile([128, 512], F32, tag="aux", name="rbb7")
                        nc.tensor.matmul(rbb[0:64, :], lhsT=ones64b[:], rhs=rtmpb[:],
                                         start=True, stop=True)
                        rb2 = rcp.tile([64, 512], F32, tag="rb2", name="rb27")
                        nc.vector.reciprocal_approx_fast(out=rb2[:], in_=rbb[0:64, :])
                        nc.vector.tensor_mul(att_dst, av[0:64, :], rb2[:])
                        return last_pt
                    # copy psum accumulator out immediately so the bank frees
                    avc = rcp.tile([65, 512], F32, tag="avc", name=f"avc{v}")
                    rbc = rcp.tile([64, 512], F32, tag="rb", name=f"rb{v}")
                    rtmp = rcp.tile([1, 512], F32, tag="rt", name=f"rt{v}")
                    nc.vector.tensor_copy(avc[:], av[:])
                    nc.vector.tensor_copy(rtmp[:], av[64:65, :])
                    rb2 = rcp.tile([64, 512], F32, tag="rb2", name=f"rb2{v}")
                    nc.gpsimd.partition_broadcast(rbc[:], rtmp[:])
                    nc.vector.reciprocal_approx_fast(out=rb2[:], in_=rbc[:])
                    nc.vector.tensor_mul(att_dst, avc[0:64, :], rb2[:])

                # ---- scope A: q proj, K0, vhead 0 with K1-7 + V dripped in ----
                pfx = tc.tile_pool(name="xin", bufs=1)
                xp = pfx.__enter__()
                scA = tc.tile_pool(name="ps_scA", bufs=2, space="PSUM")
                pssc2 = scA.__enter__()
                bgp_cm = tc.tile_pool(name="ps_bg", bufs=2, space="PSUM")
                bgp = bgp_cm.__enter__()

                Wq_sb = xp.tile([128, 512], BF16, tag="wq")
                xqT_sb = xp.tile([128, 4 * CH], BF16, tag="xq")
                # one tile per 512-seq chunk (layout [p, (k=4, 512)]) so each
                # projection group depends on exactly its own chunk's DMA
                vst = [xp.tile([128, 2048], BF16, tag=f"vs{c}", name=f"vst{c}")
                       for c in range(8)]
                kst0 = [ksp.tile([128, 2048], BF16, tag=f"ks{c}", name=f"kst{c}")
                        for c in range(8)]
                # qc0 slice + Wq first so q_proj(0) can begin immediately
                xq_d = xqT_sb[:].rearrange("p (k s) -> p k s", k=4)
                xq_s = xqT.ap().rearrange("(k p) s -> p k s", p=128)
                nc.sync.dma_start(Wq_sb[:].rearrange("p (k n) -> p k n", k=4),
                                  Wq.ap().rearrange("(k p) n -> p k n", p=128))
                nc.sync.dma_start(xq_d[:, :, 0:512], xq_s[:, :, 0:512])
                nc.sync.dma_start(
                    Wk_sb[:].rearrange("p (k n) -> p k n", k=4),
                    Wk.ap().rearrange("(k p) n -> p k n", p=128))
                nc.sync.dma_start(xq_d[:, :, 512:1024], xq_s[:, :, 512:1024])
                nc.sync.dma_start(
                    Wv_sb[:].rearrange("p (k n) -> p k n", k=4),
                    Wv.ap().rearrange("(k p) n -> p k n", p=128))
                nc.sync.dma_start(xq_d[:, :, 1024:2048], xq_s[:, :, 1024:2048])
                # K/V chunks interleaved to match vhead-0's consumption order:
                # chunk c of vhead 0 consumes kst c+1 and vst c//2
                ks_ = keysT.ap().rearrange("(k p) s -> p k s", p=128)
                vs_ = valsT.ap().rearrange("(k p) s -> p k s", p=128)
                order = [(0, 0), (0, 1), (1, 0), (0, 2), (0, 3), (1, 1),
                         (0, 4), (0, 5), (1, 2), (0, 6), (0, 7), (1, 3),
                         (1, 4), (1, 5), (1, 6), (1, 7)]
                for which, ci in order:
                    tl, src = ((kst0, ks_) if which == 0 else (vst, vs_))
                    nc.sync.dma_start(
                        tl[ci][:].rearrange("p (k s) -> p k s", k=4),
                        src[:, :, 512 * ci:512 * ci + 512])
                nc.sync.dma_start(W2_sb[:], W2.ap())

                # PE p-state warmup: matmuls run at ~1.2 GHz until the PE has
                # been continuously busy ~3us, and the first real matmul can't
                # start until its DMA lands (~12us). These throwaway 256-col
                # matmuls are emitted BEFORE q_proj so they unconditionally
                # lead the PE stream, ramping the clock while DMA streams in.
                wrm_cm = tc.tile_pool(name="ps_warm", bufs=1, space="PSUM")
                wrm = wrm_cm.__enter__()
                warm_ps = wrm.tile([64, 256], F32, tag="w", name="warm_ps")
                for _ in range(20):
                    nc.tensor.matmul(
                        warm_ps[:], lhsT=warm_sb[:, 0:64],
                        rhs=warm_sb[:, 0:256], start=True, stop=True)
                wrm_cm.__exit__(None, None, None)

                for qc in range(4):
                    q_proj(qc, bgp)
                k_proj_group(0, bgp)

                def drip_kv(c):
                    # K groups dripped just ahead of the chunks that need them
                    # (chunk c needs K group <= (2c+1)//4 <= c), V groups one
                    # chunk ahead of the lagged attn@V consumer
                    if c + 1 < 8:
                        k_proj_group(c + 1, bgp)
                    for j in (2 * c, 2 * c + 1):
                        if j < JT:
                            v_proj_group(j, bgp)

                attention_head(0, pssc2, 2, drip_kv)

                bgp_cm.__exit__(None, None, None)
                scA.__exit__(None, None, None)
                pfx.__exit__(None, None, None)

                # ---- scope B: vheads 1-7; each completed query chunk's fc is
                # dripped one dout-chunk at a time into the NEXT vhead's PE
                # slack (chunks 1-4), so every fc matmul is ready when the PE
                # reaches it; only chunk 3's fc lands in the tail ----
                with tc.tile_pool(name="ps_sc", bufs=2, space="PSUM") as pssc, \
                     tc.tile_pool(name="ps_aux", bufs=1, space="PSUM") as psaux:

                    def make_drip_fc(v):
                        if v % 2 == 1 or v < 2:
                            return None
                        qc = v // 2 - 1
                        def drip(c):
                            if 1 <= c <= 4:
                                fc_one(qc, c - 1, psaux)
                        return drip

                    for v in range(1, 2 * NV - 1):
                        attention_head(v, pssc, CHUNK, make_drip_fc(v))
                    gate_pt = attention_head(2 * NV - 1, pssc, CHUNK,
                                             aux_pool=psaux)

                    # last chunk's fc: m0-2 through the (now idle) scores pool
                    # so the tail casts don't serialize on one psum bank; the
                    # gate copy keeps the fc weight loads from entering the PE
                    # pipeline ahead of vhead 7's final attn@V matmuls
                    fcA = pssc.tile([128, 1536], F32, tag="sc", name="fcA")
                    nc.vector.tensor_copy(fcA[0:1, 0:1], gate_pt[0:1, 0:1])
                    for m in range(3):
                        nc.tensor.matmul(
                            fcA[:, 512 * m:512 * m + 512],
                            lhsT=W2_sb[:, 128 * m:128 * m + 128],
                            rhs=attTp[3][:], start=True, stop=True)
                    fcB = psaux.tile([128, 512], F32, tag="aux", name="fcB")
                    nc.tensor.matmul(fcB[:], lhsT=W2_sb[:, 384:512],
                                     rhs=attTp[3][:], start=True, stop=True)
                    nc.scalar.copy(o2p[0][:], fcA[:, 0:512])
                    nc.scalar.copy(o2p[1][:], fcA[:, 512:1024])
                    nc.vector.tensor_copy(o2p[2][:], fcA[:, 1024:1536])
                    nc.vector.tensor_copy(o2p[3][:], fcB[:])
                    for m, eng in ((0, nc.sync), (1, nc.sync),
                                   (2, nc.gpsimd), (3, nc.gpsimd)):
                        eng.dma_start(yT_d[:, m, 1536:2048], o2p[m][:])

    nc.compile()
    return nc


@functools.lru_cache(maxsize=1)
def _get_program():
    return _build_program()


def _make_in_maps(queries, keys, values, Wq, Wk, Wv, Wo, bo):
    q = np.asarray(queries, np.float32).reshape(S, D)
    kT = np.ascontiguousarray(np.asarray(keys, np.float32).reshape(S, D).T
                              ).astype(ml_dtypes.bfloat16)
    vT = np.ascontiguousarray(np.asarray(values, np.float32).reshape(S, D).T
                              ).astype(ml_dtypes.bfloat16)
    Wq = np.asarray(Wq, np.float32)
    Wk = np.asarray(Wk, np.float32)
    Wv = np.asarray(Wv, np.float32)
    # fold the two fc_out applications into one: y = a@(Wo@Wo) + (bo@Wo + bo)
    W2 = np.asarray(Wo, np.float64) @ np.asarray(Wo, np.float64)
    in_maps = []
    for c in range(NCORES):
        s, g = c // HPW, c % HPW
        in_maps.append({
            "xqT": np.ascontiguousarray(q[s * CH:(s + 1) * CH].T).astype(ml_dtypes.bfloat16),
            "keysT": kT, "valsT": vT,
            "Wq": np.ascontiguousarray(Wq[:, 128 * g:128 * g + 128]).astype(ml_dtypes.bfloat16),
            "Wk": np.ascontiguousarray(Wk[:, 128 * g:128 * g + 128]).astype(ml_dtypes.bfloat16),
            "Wv": np.ascontiguousarray(Wv[:, 128 * g:128 * g + 128]).astype(ml_dtypes.bfloat16),
            "W2": np.ascontiguousarray(W2[128 * g:128 * g + 128, :]).astype(ml_dtypes.bfloat16),
        })
    return in_maps


def _fold_bias(Wo, bo):
    Wo64 = np.asarray(Wo, np.float64)
    bo64 = np.asarray(bo, np.float64)
    return (bo64 @ Wo64 + bo64).astype(np.float32)


def _run(in_maps, **kw):
    nc = _get_program()
    return run_bass_kernel_spmd(nc, in_maps, core_ids=list(range(NCORES)), **kw)


def _gather(res, b2):
    halves = []
    for s in range(SEQW):
        acc = res.results[s * HPW]["yT"].T.astype(np.float32).copy()
        for g in range(1, HPW):
            acc += res.results[s * HPW + g]["yT"].T.astype(np.float32)
        halves.append(acc + b2)
    return np.concatenate(halves, axis=0).reshape(1, S, D)


def kernel(queries, keys, values, Wq, Wk, Wv, Wo, bo):
    res = _run(_make_in_maps(queries, keys, values, Wq, Wk, Wv, Wo, bo))
    return _gather(res, _fold_bias(Wo, bo))


def run_traced(queries, keys, values, Wq, Wk, Wv, Wo, bo):
    """Like kernel() but with NTFF profiling; returns (output, BassKernelResults)."""
    import types
    import trn_agent_boot.trn_boot as _tb
    from concourse import bass_utils
    hook = _tb._ntff_profile_via_ctypes("/opt/axon/libaxon_pjrt.so")
    mod = types.ModuleType("antenv.axon_hooks")
    mod.get_axon_ntff_profile_hook = lambda: hook
    sys.modules["antenv.axon_hooks"] = mod
    bass_utils.upload_artifacts = lambda tmpdir: tmpdir
    res = _run(_make_in_maps(queries, keys, values, Wq, Wk, Wv, Wo, bo), trace=True)
    return _gather(res, _fold_bias(Wo, bo)), res


# revision 5
# speedup vs baseline: 1.2264x; 1.1097x over previous
"""Trainium2 Bass kernel: MultiHeadSelfAttention (B=1, S=4096, D=512, H=8, DK=DV=64)
with fc_out applied twice.

Sharding: 2-way sequence x 4-way head-pair hybrid. Core c = (s, g) with
s = c//4, g = c%4 handles queries [2048s : 2048s+2048] for head pair g
(heads 2g, 2g+1):
  - Wq/Wk/Wv column-sharded by pair: each core projects only its pair's
    K^T/V over the full 4096 keys.
  - fc_out row-sharded: each core computes the partial y^T = W2[pair rows]^T
    @ att^T for its 2048 queries; the HOST sums the 4 pair-partials per
    sequence half and adds the (folded) bias. No collectives anywhere.
  - attention runs as 8 "virtual heads" (4 query chunks of 512 x head lo/hi).

Software pipeline (the key structure): vhead v's scores+exp run in window v,
its attn@V in window v+1. Window 0 emits TWO score streams (vheads 0 and 1)
plus all projections, so the ACT exp chain is never starved afterwards; the
attn@V work cascades one window behind its scores. Schedule:
  W0: sc0+sc1 + qproj + K-proj + V-proj drip        (psum: scores 6 + bg 2)
  W1: sc2+av0   W2: sc3+av1   W3: sc4+av2+fc0       (psum: scores 6 + av 1
  W4: sc5+av3+av4(aux)        W5: sc6+av5+fc1        + aux 1)
  W6: sc7+av6+av7(aux,lagged)+fc2(via scores pool)
  tail: norm7, fc3, casts, DMA out
Window walls: W0 ~43us PE-bound (ACT pre-loads 2 vheads of exp), W1-W5
ACT-paced ~15.7us, W4/W6 PE-bound (ACT catches up), tail ~5us.

Layout notes:
  - scores^T tiles [seq_k(128) x seq_q(512)] via lhsT=K^T-pair block,
    rhs=q^T slot. K^T packs head lo on rows 0-63, head hi on 64-127; q^T
    slots zero the complementary rows so K=128 matmuls never trip the PE
    HAM activity monitor (K=64 pins the clock to 1.2 GHz).
  - softmax denominator via a ones-column appended to each head's V (stride
    65): attn@V gives [65, 512] per vhead = output^T rows + exp-sum row.
  - the two fc_out applications are folded on the host (W2 = Wo@Wo,
    b2 = bo@Wo + bo); bias is added on host after the partial sum.
  - output returned TRANSPOSED ([D, 2048] bf16 partial); host sums and
    un-transposes. fc drips one dout-chunk per chunk-slot so each matmul is
    ready when the PE reaches it (wait-queue depth 4, head-of-line blocking).
  - 20 throwaway matmuls lead the PE stream to ramp the clock while the
    first DMAs land.
"""
import sys, functools
sys.path.insert(0, "/opt/trn_rl_repo")
if "/root/.axon_site" not in sys.path:
    sys.path.insert(0, "/root/.axon_site")
import numpy as np
import ml_dtypes

import concourse.bass as bass
import concourse.tile as tile
from concourse import bacc, mybir, masks
from concourse.bass_utils import run_bass_kernel_spmd

NCORES = 8
S, D, H, DK = 4096, 512, 8, 64
SEQW = 2
HPW = 4
CH = S // SEQW    # 2048 queries per core
NV = CH // 512    # 4 query chunks -> 8 virtual heads
VW = 2 * (DK + 1)           # 130: pair v row width incl. ones columns
JT = S // 128               # 32 seq_k tiles
CHUNK = 3                   # j-tiles per exp batch ([128,1536] psum)
NCH = (JT + CHUNK - 1) // CHUNK   # 11 chunks per vhead

F32 = mybir.dt.float32
BF16 = mybir.dt.bfloat16
EXP = mybir.ActivationFunctionType.Exp


def _build_program():
    nc = bacc.Bacc("TRN2", target_bir_lowering=False, debug=False,
                   num_devices=NCORES)

    xqT = nc.dram_tensor("xqT", [D, CH], BF16, kind="ExternalInput")
    keysT = nc.dram_tensor("keysT", [D, S], BF16, kind="ExternalInput")
    valsT = nc.dram_tensor("valsT", [D, S], BF16, kind="ExternalInput")
    Wq = nc.dram_tensor("Wq", [D, 128], BF16, kind="ExternalInput")
    Wk = nc.dram_tensor("Wk", [D, 128], BF16, kind="ExternalInput")
    Wv = nc.dram_tensor("Wv", [D, 128], BF16, kind="ExternalInput")
    W2 = nc.dram_tensor("W2", [128, D], BF16, kind="ExternalInput")
    yT = nc.dram_tensor("yT", [D, CH], BF16, kind="ExternalOutput")
    yT_d = yT.ap().rearrange("(m p) f -> p m f", m=4, p=128)

    with tile.TileContext(nc) as tc:
        with tc.tile_pool(name="persist", bufs=1) as pp, \
             tc.tile_pool(name="kv", bufs=1) as kvp, \
             tc.tile_pool(name="pt", bufs=23) as ptp, \
             tc.tile_pool(name="rc", bufs=2) as rcp:

            W2_sb = pp.tile([128, 512], BF16, tag="w2")
            Wk_sb = pp.tile([128, 512], BF16, tag="wk")
            Wv_sb = pp.tile([128, 512], BF16, tag="wv")
            ones64b = pp.tile([1, 64], BF16, tag="on")
            warm_sb = pp.tile([128, 256], BF16, tag="warm")
            o2p = [pp.tile([128, 512], BF16, tag=f"o2{m}", name=f"o2_{m}")
                   for m in range(4)]
            qTz_sb = pp.tile([128, 2 * NV * 512], BF16, tag="qt")
            attTp = [pp.tile([128, 512], BF16, tag=f"att{p}", name=f"attT{p}")
                     for p in range(4)]
            KTp = kvp.tile([128, S], BF16, tag="kt")
            V_sb = kvp.tile([128, JT * VW], BF16, tag="v")

            nc.vector.memset(warm_sb[:], 0.0)
            nc.vector.memset(qTz_sb[:], 0.0)
            nc.vector.memset(ones64b[:], 1.0)
            nc.gpsimd.memset(
                V_sb[:].rearrange("p (j h x) -> p j h x", j=JT, h=2, x=DK + 1)
                [:, :, :, DK:DK + 1], 1.0)

            # scores psum pool spans all windows (6 banks)
            scp_cm = tc.tile_pool(name="ps_sc", bufs=2, space="PSUM")
            scp = scp_cm.__enter__()

            # ---- window 0 scope: staging + projection psums ----
            ksp_cm = tc.tile_pool(name="kstage", bufs=1)
            ksp = ksp_cm.__enter__()
            xin_cm = tc.tile_pool(name="xin", bufs=1)
            xp = xin_cm.__enter__()
            bgp_cm = tc.tile_pool(name="ps_bg", bufs=2, space="PSUM")
            bgp = bgp_cm.__enter__()

            Wq_sb = xp.tile([128, 512], BF16, tag="wq")
            xqT_sb = xp.tile([128, 4 * CH], BF16, tag="xq")
            vst = [xp.tile([128, 2048], BF16, tag=f"vs{c}", name=f"vst{c}")
                   for c in range(8)]
            kst0 = [ksp.tile([128, 2048], BF16, tag=f"ks{c}", name=f"kst{c}")
                    for c in range(8)]

            xq_d = xqT_sb[:].rearrange("p (k s) -> p k s", k=4)
            xq_s = xqT.ap().rearrange("(k p) s -> p k s", p=128)
            ks_ = keysT.ap().rearrange("(k p) s -> p k s", p=128)
            vs_ = valsT.ap().rearrange("(k p) s -> p k s", p=128)
            nc.sync.dma_start(Wq_sb[:].rearrange("p (k n) -> p k n", k=4),
                              Wq.ap().rearrange("(k p) n -> p k n", p=128))
            nc.sync.dma_start(xq_d[:, :, 0:1024], xq_s[:, :, 0:1024])
            nc.sync.dma_start(
                Wk_sb[:].rearrange("p (k n) -> p k n", k=4),
                Wk.ap().rearrange("(k p) n -> p k n", p=128))
            nc.sync.dma_start(
                kst0[0][:].rearrange("p (k s) -> p k s", k=4), ks_[:, :, 0:512])
            nc.sync.dma_start(xq_d[:, :, 1024:2048], xq_s[:, :, 1024:2048])
            nc.sync.dma_start(
                Wv_sb[:].rearrange("p (k n) -> p k n", k=4),
                Wv.ap().rearrange("(k p) n -> p k n", p=128))
            # K chunks lead 2:1 early (K drip consumes 1/chunk, V 3/chunk but
            # V groups j map to vst[j//4] so vst drains 4x slower per tile)
            order = [(0, 1), (1, 0), (0, 2), (1, 1), (0, 3), (1, 2),
                     (0, 4), (1, 3), (0, 5), (1, 4), (0, 6), (1, 5),
                     (0, 7), (1, 6), (1, 7)]
            for which, ci in order:
                tl, src = ((kst0, ks_) if which == 0 else (vst, vs_))
                nc.sync.dma_start(
                    tl[ci][:].rearrange("p (k s) -> p k s", k=4),
                    src[:, :, 512 * ci:512 * ci + 512])
            nc.sync.dma_start(W2_sb[:], W2.ap())

            # PE p-state warmup through the bg pool (throwaway matmuls)
            for i in range(20):
                wp = bgp.tile([128, 512], F32, tag="bg", name=f"warm{i}")
                nc.tensor.matmul(wp[0:64, 0:256], lhsT=warm_sb[:, 0:64],
                                 rhs=warm_sb[:, 0:256], start=True, stop=True)

            def q_proj(qc, pool):
                ps = pool.tile([128, 512], F32, tag="bg", name=f"qp{qc}")
                for k in range(4):
                    nc.tensor.matmul(
                        ps[:], lhsT=Wq_sb[:, 128 * k:128 * k + 128],
                        rhs=xqT_sb[:, 2048 * k + 512 * qc:2048 * k + 512 * qc + 512],
                        start=(k == 0), stop=(k == 3))
                nc.vector.tensor_copy(
                    qTz_sb[0:64, 512 * (2 * qc):512 * (2 * qc) + 512], ps[0:64, :])
                nc.vector.tensor_copy(
                    qTz_sb[64:128, 512 * (2 * qc + 1):512 * (2 * qc + 1) + 512],
                    ps[64:128, :])

            def v_proj_group(j):
                ps = bgp.tile([128, 512], F32, tag="bg", name=f"vp{j}")
                vt, jj = vst[j // 4], j % 4
                for k in range(4):
                    nc.tensor.matmul(
                        ps[0:128, 0:128],
                        lhsT=vt[:, 512 * k + 128 * jj:512 * k + 128 * jj + 128],
                        rhs=Wv_sb[:, 128 * k:128 * k + 128],
                        start=(k == 0), stop=(k == 3))
                dst = V_sb[:, VW * j:VW * j + VW].rearrange(
                    "p (h x) -> p h x", h=2, x=DK + 1)[:, :, 0:DK]
                nc.vector.tensor_copy(
                    dst, ps[0:128, 0:128].rearrange("p (h x) -> p h x", h=2, x=DK))

            def k_proj_group(sc):
                ps = bgp.tile([128, 512], F32, tag="bg", name=f"kp{sc}")
                for k in range(4):
                    nc.tensor.matmul(
                        ps[:], lhsT=Wk_sb[:, 128 * k:128 * k + 128],
                        rhs=kst0[sc][:, 512 * k:512 * k + 512],
                        start=(k == 0), stop=(k == 3))
                nc.vector.tensor_copy(KTp[:, 512 * sc:512 * sc + 512], ps[:])

            pts = {v: [] for v in range(2 * NV)}   # per-vhead pt tiles

            def chunk_js(c):
                return list(range(CHUNK * c, min(CHUNK * c + CHUNK, JT)))

            def scores_chunk(v, c):
                js = chunk_js(c)
                ps = scp.tile([128, 512 * CHUNK], F32, tag="sc",
                              name=f"sc{v}_{c}")
                pt = ptp.tile([128, 512 * CHUNK], BF16, tag="pt",
                              name=f"pt{v}_{c}")
                q_ap = qTz_sb[:, 512 * v:512 * v + 512]
                for i, j in enumerate(js):
                    nc.tensor.matmul(
                        ps[:, 512 * i:512 * i + 512],
                        lhsT=KTp[:, 128 * j:128 * j + 128],
                        rhs=q_ap, start=True, stop=True)
                w = 512 * len(js)
                nc.scalar.activation(pt[:, 0:w], ps[:, 0:w], EXP, scale=0.125)
                pts[v].append(pt)

            def attn_batch(v, av, c):
                voff = 65 * (v % 2)
                for i, j in enumerate(chunk_js(c)):
                    nc.tensor.matmul(
                        av[0:65, :],
                        lhsT=V_sb[:, VW * j + voff:VW * j + voff + 65],
                        rhs=pts[v][c][:, 512 * i:512 * i + 512],
                        start=(j == 0), stop=(j == JT - 1))

            def norm_std(v, av):
                qc, hl = v // 2, v % 2
                att_dst = attTp[qc][64 * hl:64 * hl + 64, :]
                avc = rcp.tile([65, 512], F32, tag="avc", name=f"avc{v}")
                rbc = rcp.tile([64, 512], F32, tag="rb", name=f"rb{v}")
                rtmp = rcp.tile([1, 512], F32, tag="rt", name=f"rt{v}")
                nc.vector.tensor_copy(avc[:], av[0:65, :])
                nc.vector.tensor_copy(rtmp[:], av[64:65, :])
                rb2 = rcp.tile([64, 512], F32, tag="rb2", name=f"rb2{v}")
                nc.gpsimd.partition_broadcast(rbc[:], rtmp[:])
                nc.vector.reciprocal_approx_fast(out=rb2[:], in_=rbc[:])
                nc.vector.tensor_mul(att_dst, avc[0:64, :], rb2[:])

            # ---- window 0: sc0 + sc1 + all projections ----
            q_proj(0, bgp)
            q_proj(1, bgp)
            k_proj_group(0)
            for c in range(NCH):
                if c == 0:
                    q_proj(2, bgp)
                if c == 1:
                    q_proj(3, bgp)
                if c + 1 < 8:
                    k_proj_group(c + 1)
                for j in (3 * c, 3 * c + 1, 3 * c + 2):
                    if j < JT:
                        v_proj_group(j)
                scores_chunk(0, c)
                scores_chunk(1, c)

            bgp_cm.__exit__(None, None, None)
            xin_cm.__exit__(None, None, None)
            ksp_cm.__exit__(None, None, None)

            # ---- windows 1-6 + tail ----
            with tc.tile_pool(name="ps_av", bufs=1, space="PSUM") as psav, \
                 tc.tile_pool(name="ps_aux", bufs=1, space="PSUM") as psaux:

                def fc_emit(qc, m):
                    ps = psaux.tile([128, 512], F32, tag="aux",
                                    name=f"fc{qc}_{m}")
                    nc.tensor.matmul(
                        ps[:], lhsT=W2_sb[:, 128 * m:128 * m + 128],
                        rhs=attTp[qc][:], start=True, stop=True)
                    nc.vector.tensor_copy(o2p[m][:], ps[:])
                    eng = nc.sync if m % 2 == 0 else nc.gpsimd
                    eng.dma_start(yT_d[:, m, 512 * qc:512 * qc + 512], o2p[m][:])

                # (scores_vhead, [(av_vhead, pool)], fc_qc, lagged_self_av)
                def av_tile(v, pool):
                    return pool.tile([128, 512], F32, tag="aux", name=f"av{v}") \
                        if pool is psaux else \
                        pool.tile([65, 512], F32, tag="av", name=f"av{v}")

                WIN = [
                    (2, [(0, psav)], None, False),
                    (3, [(1, psav)], None, False),
                    (4, [(2, psav)], 0, False),
                    (5, [(3, psav), (4, psaux)], None, False),
                    (6, [(5, psav)], 1, False),
                    (7, [(6, psav)], 2, True),
                ]
                for sv, avl, fcqc, lagged in WIN:
                    avts = [(v, av_tile(v, pool)) for v, pool in avl]
                    lag_av = av_tile(sv, psaux) if lagged else None
                    pend = None
                    for c in range(NCH):
                        for v, avt in avts:
                            attn_batch(v, avt, c)
                        if fcqc is not None and 1 <= c <= 4 and fcqc != 2:
                            fc_emit(fcqc, c - 1)
                        if fcqc == 2 and c == 1:
                            # aux is occupied by av7 this window: route fc2
                            # through the (rotating) scores pool instead
                            fcC = scp.tile([128, 1536], F32, tag="sc", name="fcC")
                            for m in range(3):
                                nc.tensor.matmul(
                                    fcC[:, 512 * m:512 * m + 512],
                                    lhsT=W2_sb[:, 128 * m:128 * m + 128],
                                    rhs=attTp[2][:], start=True, stop=True)
                            for m in range(3):
                                nc.vector.tensor_copy(o2p[m][:],
                                                      fcC[:, 512 * m:512 * m + 512])
                                eng = nc.sync if m % 2 == 0 else nc.gpsimd
                                eng.dma_start(yT_d[:, m, 1024:1536], o2p[m][:])
                        if fcqc == 2 and c == 3:
                            fcD = scp.tile([128, 1536], F32, tag="sc", name="fcD")
                            nc.tensor.matmul(
                                fcD[:, 0:512], lhsT=W2_sb[:, 384:512],
                                rhs=attTp[2][:], start=True, stop=True)
                            nc.vector.tensor_copy(o2p[3][:], fcD[:, 0:512])
                            nc.gpsimd.dma_start(yT_d[:, 3, 1024:1536], o2p[3][:])
                        scores_chunk(sv, c)
                        if lagged:
                            if pend is not None:
                                attn_batch(sv, lag_av, pend)
                            pend = c
                    for v, avt in avts:
                        norm_std(v, avt)
                    if lagged:
                        attn_batch(sv, lag_av, pend)

                # ---- tail: norm7 (fast), fc3, casts, DMA ----
                v7 = 2 * NV - 1
                att_dst = attTp[3][64:128, :]
                rtmpb = rcp.tile([1, 512], BF16, tag="rt", name="rt7")
                nc.vector.tensor_copy(rtmpb[:], lag_av[64:65, :])
                rbb = psav.tile([65, 512], F32, tag="av", name="rbb7")
                nc.tensor.matmul(rbb[0:64, :], lhsT=ones64b[:], rhs=rtmpb[:],
                                 start=True, stop=True)
                rb2 = rcp.tile([64, 512], F32, tag="rb2", name="rb27")
                nc.vector.reciprocal_approx_fast(out=rb2[:], in_=rbb[0:64, :])
                nc.vector.tensor_mul(att_dst, lag_av[0:64, :], rb2[:])

                gate_pt = pts[v7][NCH - 1]
                fcA = scp.tile([128, 1536], F32, tag="sc", name="fcA")
                nc.vector.tensor_copy(fcA[0:1, 0:1], gate_pt[0:1, 0:1])
                for m in range(3):
                    nc.tensor.matmul(
                        fcA[:, 512 * m:512 * m + 512],
                        lhsT=W2_sb[:, 128 * m:128 * m + 128],
                        rhs=attTp[3][:], start=True, stop=True)
                fcB = psaux.tile([128, 512], F32, tag="aux", name="fcB")
                nc.tensor.matmul(fcB[:], lhsT=W2_sb[:, 384:512],
                                 rhs=attTp[3][:], start=True, stop=True)
                nc.scalar.copy(o2p[0][:], fcA[:, 0:512])
                nc.scalar.copy(o2p[1][:], fcA[:, 512:1024])
                nc.vector.tensor_copy(o2p[2][:], fcA[:, 1024:1536])
                nc.vector.tensor_copy(o2p[3][:], fcB[:])
                for m, eng in ((0, nc.sync), (1, nc.sync),
                               (2, nc.gpsimd), (3, nc.gpsimd)):
                    eng.dma_start(yT_d[:, m, 1536:2048], o2p[m][:])

            scp_cm.__exit__(None, None, None)

    nc.compile()
    return nc


@functools.lru_cache(maxsize=1)
def _get_program():
    return _build_program()


def _make_in_maps(queries, keys, values, Wq, Wk, Wv, Wo, bo):
    q = np.asarray(queries, np.float32).reshape(S, D)
    kT = np.ascontiguousarray(np.asarray(keys, np.float32).reshape(S, D).T
                              ).astype(ml_dtypes.bfloat16)
    vT = np.ascontiguousarray(np.asarray(values, np.float32).reshape(S, D).T
                              ).astype(ml_dtypes.bfloat16)
    Wq = np.asarray(Wq, np.float32)
    Wk = np.asarray(Wk, np.float32)
    Wv = np.asarray(Wv, np.float32)
    W2 = np.asarray(Wo, np.float64) @ np.asarray(Wo, np.float64)
    in_maps = []
    for c in range(NCORES):
        s, g = c // HPW, c % HPW
        in_maps.append({
            "xqT": np.ascontiguousarray(q[s * CH:(s + 1) * CH].T).astype(ml_dtypes.bfloat16),
            "keysT": kT, "valsT": vT,
            "Wq": np.ascontiguousarray(Wq[:, 128 * g:128 * g + 128]).astype(ml_dtypes.bfloat16),
            "Wk": np.ascontiguousarray(Wk[:, 128 * g:128 * g + 128]).astype(ml_dtypes.bfloat16),
            "Wv": np.ascontiguousarray(Wv[:, 128 * g:128 * g + 128]).astype(ml_dtypes.bfloat16),
            "W2": np.ascontiguousarray(W2[128 * g:128 * g + 128, :]).astype(ml_dtypes.bfloat16),
        })
    return in_maps


def _fold_bias(Wo, bo):
    Wo64 = np.asarray(Wo, np.float64)
    bo64 = np.asarray(bo, np.float64)
    return (bo64 @ Wo64 + bo64).astype(np.float32)


def _run(in_maps, **kw):
    nc = _get_program()
    return run_bass_kernel_spmd(nc, in_maps, core_ids=list(range(NCORES)), **kw)


def _gather(res, b2):
    halves = []
    for s in range(SEQW):
        acc = res.results[s * HPW]["yT"].T.astype(np.float32).copy()
        for g in range(1, HPW):
            acc += res.results[s * HPW + g]["yT"].T.astype(np.float32)
        halves.append(acc + b2)
    return np.concatenate(halves, axis=0).reshape(1, S, D)


def kernel(queries, keys, values, Wq, Wk, Wv, Wo, bo):
    res = _run(_make_in_maps(queries, keys, values, Wq, Wk, Wv, Wo, bo))
    return _gather(res, _fold_bias(Wo, bo))


def run_traced(queries, keys, values, Wq, Wk, Wv, Wo, bo):
    """Like kernel() but with NTFF profiling; returns (output, BassKernelResults)."""
    import types
    import trn_agent_boot.trn_boot as _tb
    from concourse import bass_utils
    hook = _tb._ntff_profile_via_ctypes("/opt/axon/libaxon_pjrt.so")
    mod = types.ModuleType("antenv.axon_hooks")
    mod.get_axon_ntff_profile_hook = lambda: hook
    sys.modules["antenv.axon_hooks"] = mod
    bass_utils.upload_artifacts = lambda tmpdir: tmpdir
    res = _run(_make_in_maps(queries, keys, values, Wq, Wk, Wv, Wo, bo), trace=True)
    return _gather(res, _fold_bias(Wo, bo)), res
